# revision 1
# baseline (speedup 1.0000x reference)
"""Trainium2 Bass kernel for AudioAdapterAttnProcessor.

Reference computation (B=4, S=4096, D=1024, H=16, HD=64, C=768,
S_TXT=77, S_AUD=16):
    q = (hidden @ Wq)                                  [B, S, H, HD]
    base  = softmax(q k_t^T / 8) v_t   (text cross-attn, k/v from encoder)
    audio = softmax(q k_a^T / 8) v_a   (audio cross-attn)
    out = concat_heads(base + audio) @ Wo + bo

Sharding: (batch x seq-half) -> 8 cores; each core handles one batch's
2048 queries for all 16 heads.  No collectives: every query row of the
output depends only on its own hidden row (cross-attention to 93 fixed
keys per batch), so the gather is a pure concatenation.

Per-core device program (everything transposed so PE contracts naturally):
    qT   = Wq^T @ hiddenT                (hiddenT fed pre-transposed, bf16)
    s    = q_h^T.T @ kcatT_h             natural scores [128q, 93keys]
    p    = exp(s/8) (fused scale); denominators via pair-batched DVE
           segmented reduces + reciprocal; normalize via broadcast mult
    pT   = PE-transpose(p)               [93, 128] bf16 per (head, q-tile)
    pv   = v_h^T.T @ pT                  -> attn_outT [64, 512] per head
    out  = attn_T.T @ Wo                 natural [128q, 1024] fp32 -> DRAM
bo is added on the host during the gather (it is per-output-feature,
which is the free dim of the natural-layout output).

The PE stream is software-pipelined two ways: the q-projection of chunk
c+1 AND the out-projection of chunk c-1 are interleaved between the
scores/transpose/PV matmuls of chunk c (pv_lag pair-slots behind the
scores), so the PE never waits on the PE->ACT->DVE softmax round-trip
(~4us per head pair).

Designs tried and rejected on hardware: transposed scores with GPSIMD
partition_all_reduce denominators (each reduce costs ~3.6us on real HW
vs 0.8us in the cost model — 64 of them added ~230us/rep), and fp8
DoubleRow projections (4-8% rel err vs the 2% tolerance).
"""

import sys

sys.path.insert(0, "/opt/trn_rl_repo")

from contextlib import ExitStack

import numpy as np
import ml_dtypes

import concourse.bass as bass
import concourse.mybir as mybir
import concourse.tile as tile
from concourse import bacc
from concourse.masks import make_identity

BF16 = ml_dtypes.bfloat16

B, S, D = 4, 4096, 1024
S_TXT, S_AUD = 77, 16
C = 768
H = 16
HD = 64
NK = S_TXT + S_AUD  # 93 keys after concat
P = 128
SCALE = 1.0 / np.sqrt(HD)  # 0.125

N_CORES = 8
SEQ_PER_CORE = S // 2  # 2048
CHUNK = 512
N_CHUNKS = SEQ_PER_CORE // CHUNK  # 4
KT = D // P  # 8 contraction tiles
QT_PER_CHUNK = CHUNK // P  # 4
NG = H // 2  # 8 head pairs == 8 mt groups == 8 outproj groups


DEFAULT_CFG = dict(hidden=3, qT=2, puT=3, dall=3, pnT=3, expT=2, attn_T=2,
                   out_sb=3, accps=4, sps=2, tps=1, vps=1, pv_lag=1,
                   prio_qT=0, prio_at=0, prio_expT=0, qT_dve=False, at_dve=False,
                   expT_dve=False, ob_dve=True, out_bf16=True, out_dma_act=False,
                   pu_bf16=True)


def build_bass(cfg=None, reps=1):
    """Build the SPMD single-core Bass program (same program on all 8 cores).

    reps > 1 repeats the whole computation back-to-back inside the NEFF
    (same inputs -> same outputs); used only for slope-based timing.
    """
    cfg = {**DEFAULT_CFG, **(cfg or {})}
    nc = bacc.Bacc("TRN2", target_bir_lowering=False, debug=False, num_devices=N_CORES)

    ht_d = nc.dram_tensor("ht", [P, KT * SEQ_PER_CORE], mybir.dt.bfloat16, kind="ExternalInput")
    wq_d = nc.dram_tensor("wq", [P, KT * D], mybir.dt.bfloat16, kind="ExternalInput")
    wo_d = nc.dram_tensor("wo", [P, KT * D], mybir.dt.bfloat16, kind="ExternalInput")
    kc_d = nc.dram_tensor("kc", [P, KT * NK], mybir.dt.bfloat16, kind="ExternalInput")
    v_d = nc.dram_tensor("v", [P, D], mybir.dt.bfloat16, kind="ExternalInput")
    out_dt = mybir.dt.bfloat16 if cfg["out_bf16"] else mybir.dt.float32
    out_d = nc.dram_tensor("out", [SEQ_PER_CORE, D], out_dt, kind="ExternalOutput")

    ht_view = ht_d[:].rearrange("p (k q) -> p k q", k=KT)  # [128, 8, 2048]

    import contextlib

    with tile.TileContext(nc) as tc, ExitStack() as ctx:
        def gate_copy(dst, src, prio, on_dve):
            """PSUM->SBUF copy that gates PE work; optionally boosted/moved."""
            cm = (tc.high_priority(None if prio < 0 else prio)
                  if prio else contextlib.nullcontext())
            with cm:
                if on_dve:
                    nc.vector.tensor_copy(dst, src)
                else:
                    nc.scalar.copy(dst, src)

        wpool = ctx.enter_context(tc.tile_pool(name="weights", bufs=1))
        hpool = ctx.enter_context(tc.tile_pool(name="hidden", bufs=cfg["hidden"]))
        qpool = ctx.enter_context(tc.tile_pool(name="qT", bufs=cfg["qT"]))
        pupool = ctx.enter_context(tc.tile_pool(name="pu", bufs=cfg["puT"]))
        dpool = ctx.enter_context(tc.tile_pool(name="ds", bufs=cfg["dall"]))
        pnpool = ctx.enter_context(tc.tile_pool(name="pn", bufs=cfg["pnT"]))
        epool = ctx.enter_context(tc.tile_pool(name="expT", bufs=cfg["expT"]))
        atpool = ctx.enter_context(tc.tile_pool(name="attn_T", bufs=cfg["attn_T"]))
        opool = ctx.enter_context(tc.tile_pool(name="out_sb", bufs=cfg["out_sb"]))

        accps = ctx.enter_context(tc.tile_pool(name="accps", bufs=cfg["accps"], space="PSUM"))
        sps = ctx.enter_context(tc.tile_pool(name="sps", bufs=cfg["sps"], space="PSUM"))
        tps = ctx.enter_context(tc.tile_pool(name="tps", bufs=cfg["tps"], space="PSUM"))
        vps = ctx.enter_context(tc.tile_pool(name="vps", bufs=cfg["vps"], space="PSUM"))

        # One-time loads
        wq_t = wpool.tile([P, KT * D], mybir.dt.bfloat16)
        wo_t = wpool.tile([P, KT * D], mybir.dt.bfloat16)
        kc_t = wpool.tile([P, KT * NK], mybir.dt.bfloat16)
        v_t = wpool.tile([P, D], mybir.dt.bfloat16)
        ident = wpool.tile([P, P], mybir.dt.bfloat16)
        # DMA issue order matters: the HWDGE queue drains in order, so put
        # everything the first q-projection needs ahead of the 2MB wo load.
        # Per-k-tile splits let matmuls start on sub-tile deps.
        ht_t0 = hpool.tile([P, KT, CHUNK], mybir.dt.bfloat16, tag="ht")
        for kt in range(KT):
            nc.sync.dma_start(wq_t[:, kt * D : (kt + 1) * D], wq_d[:, kt * D : (kt + 1) * D])
            nc.sync.dma_start(ht_t0[:, kt, :], ht_view[:, kt, 0:CHUNK])
        nc.sync.dma_start(kc_t[:], kc_d[:])
        nc.sync.dma_start(v_t[:], v_d[:])
        for kt in range(KT):
            nc.sync.dma_start(wo_t[:, kt * D : (kt + 1) * D], wo_d[:, kt * D : (kt + 1) * D])
        make_identity(nc, ident[:])

        def emit_ht(pos_chunk, tile_=None):
            ht_t = tile_ or hpool.tile([P, KT, CHUNK], mybir.dt.bfloat16, tag="ht")
            if tile_ is None:
                nc.sync.dma_start(
                    ht_t[:], ht_view[:, :, pos_chunk * CHUNK : (pos_chunk + 1) * CHUNK]
                )
            return ht_t

        def emit_qproj_mt(ht_t, qT_t, mt):
            qp = accps.tile([P, CHUNK], mybir.dt.float32, tag="acc")
            for kt in range(KT):
                nc.tensor.matmul(
                    qp[:],
                    lhsT=wq_t[:, kt * D + mt * P : kt * D + (mt + 1) * P],
                    rhs=ht_t[:, kt, :],
                    start=(kt == 0),
                    stop=(kt == KT - 1),
                )
            gate_copy(qT_t[:, mt, :], qp[:], cfg["prio_qT"], cfg["qT_dve"])

        def emit_scores_pair(qT_t, g):
            """Natural-layout scores + softmax for both heads of pair g.

            Per pair: 8 PE matmuls (ap=93), 2 ACT exps (one per head, into a
            shared pair tile), then pair-batched DVE: 2 segmented reduces,
            1 reciprocal, 2 normalizes.  Returns the bf16 probs tile
            pn [P, 2, QT, NK] (natural layout, q on partitions).
            """
            pu_dt = mybir.dt.bfloat16 if cfg["pu_bf16"] else mybir.dt.float32
            pu = pupool.tile([P, 2, QT_PER_CHUNK, NK], pu_dt)
            for hh in range(2):
                off = hh * HD
                sp = sps.tile([P, QT_PER_CHUNK * NK], mybir.dt.float32)
                for qt in range(QT_PER_CHUNK):
                    nc.tensor.matmul(
                        sp[:, qt * NK : (qt + 1) * NK],
                        lhsT=qT_t[off : off + HD, g, qt * P : (qt + 1) * P],
                        rhs=kc_t[off : off + HD, g * NK : (g + 1) * NK],
                        start=True,
                        stop=True,
                    )
                nc.scalar.activation(
                    pu[:, hh, :, :].rearrange("p q k -> p (q k)"),
                    sp[:],
                    mybir.ActivationFunctionType.Exp,
                    scale=float(SCALE),
                )
            # 16 denominators per pair: 2 heads x 4 q-tiles x (text, audio)
            ds = dpool.tile([P, 2, QT_PER_CHUNK, 2], mybir.dt.float32)
            nc.vector.reduce_sum(
                ds[:, :, :, 0], pu[:, :, :, 0:S_TXT], axis=mybir.AxisListType.X
            )
            nc.vector.reduce_sum(
                ds[:, :, :, 1], pu[:, :, :, S_TXT:NK], axis=mybir.AxisListType.X
            )
            nc.vector.reciprocal(
                ds[:].rearrange("p a b c -> p (a b c)"),
                ds[:].rearrange("p a b c -> p (a b c)"),
            )
            pn = pnpool.tile([P, 2, QT_PER_CHUNK, NK], mybir.dt.bfloat16)
            nc.vector.tensor_tensor(
                pn[:, :, :, 0:S_TXT],
                pu[:, :, :, 0:S_TXT],
                ds[:, :, :, 0, None].to_broadcast([P, 2, QT_PER_CHUNK, S_TXT]),
                mybir.AluOpType.mult,
            )
            nc.vector.tensor_tensor(
                pn[:, :, :, S_TXT:NK],
                pu[:, :, :, S_TXT:NK],
                ds[:, :, :, 1, None].to_broadcast([P, 2, QT_PER_CHUNK, S_AUD]),
                mybir.AluOpType.mult,
            )
            return pn

        def emit_pv_pair(g, pn, at_t):
            """PE transposes of the pair's probs, one PSUM->SBUF copy, then
            the two PV matmuls into the pair's attn_T rows."""
            tpp = tps.tile([NK, 2, CHUNK], mybir.dt.bfloat16)
            for hh in range(2):
                for qt in range(QT_PER_CHUNK):
                    nc.tensor.transpose(
                        tpp[:, hh, qt * P : (qt + 1) * P],
                        pn[:, hh, qt, :],
                        ident[:],
                    )
            expT = epool.tile([NK, 2, CHUNK], mybir.dt.bfloat16)
            gate_copy(expT[:], tpp[:], cfg["prio_expT"], cfg["expT_dve"])
            pvp = vps.tile([P, CHUNK], mybir.dt.float32)
            for hh in range(2):
                h = 2 * g + hh
                off = hh * HD
                nc.tensor.matmul(
                    pvp[off : off + HD, :],
                    lhsT=v_t[0:NK, h * HD : (h + 1) * HD],
                    rhs=expT[0:NK, hh, :],
                    start=True,
                    stop=True,
                )
            # attn_T tile g holds heads 2g (rows 0-63) and 2g+1 (64-127)
            gate_copy(at_t[:, g, :], pvp[:], cfg["prio_at"], cfg["at_dve"])

        def emit_outproj_group(c, at_t, j):
            qt, nb = j // 2, j % 2
            op = accps.tile([P, CHUNK], mybir.dt.float32, tag="acc")
            for kt in range(KT):
                nc.tensor.matmul(
                    op[:],
                    lhsT=at_t[:, kt, qt * P : (qt + 1) * P],
                    rhs=wo_t[:, kt * D + nb * CHUNK : kt * D + (nb + 1) * CHUNK],
                    start=(kt == 0),
                    stop=(kt == KT - 1),
                )
            ob = opool.tile([P, CHUNK], out_dt)
            gate_copy(ob[:], op[:], 0, cfg["ob_dve"])
            out_q = nc.scalar if cfg["out_dma_act"] else nc.sync
            out_q.dma_start(
                out_d[
                    c * CHUNK + qt * P : c * CHUNK + (qt + 1) * P,
                    nb * CHUNK : (nb + 1) * CHUNK,
                ],
                ob[:],
            )

        chunks = [c for _ in range(reps) for c in range(N_CHUNKS)]
        n = len(chunks)
        ht_tiles = {0: ht_t0}
        if n > 1:
            ht_tiles[1] = emit_ht(chunks[1])

        # Prologue: q-projection of the first chunk.
        qT_t = qpool.tile([P, KT, CHUNK], mybir.dt.bfloat16)
        for mt in range(KT):
            emit_qproj_mt(ht_tiles[0], qT_t, mt)

        at_prev = None
        c_prev = None
        for i, c in enumerate(chunks):
            if i + 2 < n:
                ht_tiles[i + 2] = emit_ht(chunks[i + 2])
            ht_nxt = ht_tiles.get(i + 1)
            ht_tiles.pop(i, None)
            qT_next = (qpool.tile([P, KT, CHUNK], mybir.dt.bfloat16, name="qT_next")
                       if ht_nxt is not None else None)
            at_t = atpool.tile([P, KT, CHUNK], mybir.dt.bfloat16, tag="at")
            # PV trails scoresT by PV_LAG pair-slots so the 5-engine softmax
            # chain latency (~5us) hides under interleaved outproj/qproj work.
            lag = cfg["pv_lag"]
            pn_q = {}
            for g in range(NG):
                pn_q[g] = emit_scores_pair(qT_t, g)
                if g >= lag:
                    if at_prev is not None:
                        emit_outproj_group(c_prev, at_prev, g - lag)
                    if qT_next is not None:
                        emit_qproj_mt(ht_nxt, qT_next, g - lag)
                    emit_pv_pair(g - lag, pn_q.pop(g - lag), at_t)
            for g in range(NG - lag, NG):
                if at_prev is not None:
                    emit_outproj_group(c_prev, at_prev, g)
                if qT_next is not None:
                    emit_qproj_mt(ht_nxt, qT_next, g)
                emit_pv_pair(g, pn_q.pop(g), at_t)
            at_prev, c_prev = at_t, c
            if qT_next is not None:
                qT_t = qT_next
        # Epilogue: outproj of the final chunk.
        for j in range(2 * QT_PER_CHUNK):
            emit_outproj_group(c_prev, at_prev, j)

    nc.compile()
    return nc


def _host_prep(hidden_states, encoder_hidden_states, audio_hidden_states,
               Wq, Wk, Wv, Wk_audio, Wv_audio, Wo):
    """Build the per-core input maps (all layouts pre-arranged on host)."""
    wq_sb = np.ascontiguousarray(
        Wq.reshape(KT, P, D).transpose(1, 0, 2).reshape(P, KT * D)
    ).astype(BF16)
    wo_sb = np.ascontiguousarray(
        Wo.reshape(KT, P, D).transpose(1, 0, 2).reshape(P, KT * D)
    ).astype(BF16)

    in_maps = []
    for b in range(B):
        # kv projections for this batch: tiny, done on host
        k_full = np.concatenate(
            [encoder_hidden_states[b] @ Wk, audio_hidden_states[b] @ Wk_audio], axis=0
        )  # [93, 1024]
        v_full = np.concatenate(
            [encoder_hidden_states[b] @ Wv, audio_hidden_states[b] @ Wv_audio], axis=0
        )  # [93, 1024]
        kc_sb = np.ascontiguousarray(
            k_full.T.reshape(KT, P, NK).transpose(1, 0, 2).reshape(P, KT * NK)
        ).astype(BF16)
        v_sb = np.zeros((P, D), dtype=BF16)
        v_sb[:NK] = v_full.astype(BF16)

        for half in range(2):
            rows = hidden_states[b, half * SEQ_PER_CORE : (half + 1) * SEQ_PER_CORE]
            ht_sb = np.ascontiguousarray(
                rows.T.reshape(KT, P, SEQ_PER_CORE)
                .transpose(1, 0, 2)
                .reshape(P, KT * SEQ_PER_CORE)
            ).astype(BF16)
            in_maps.append(
                {"ht": ht_sb, "wq": wq_sb, "wo": wo_sb, "kc": kc_sb, "v": v_sb}
            )
    return in_maps


_NC_CACHE = {}


def get_nc():
    if "nc" not in _NC_CACHE:
        _NC_CACHE["nc"] = build_bass()
    return _NC_CACHE["nc"]


def kernel(hidden_states, encoder_hidden_states, audio_hidden_states,
           Wq, Wk, Wv, Wk_audio, Wv_audio, Wo, bo):
    from concourse import bass_utils

    hidden_states = np.asarray(hidden_states, dtype=np.float32)
    encoder_hidden_states = np.asarray(encoder_hidden_states, dtype=np.float32)
    audio_hidden_states = np.asarray(audio_hidden_states, dtype=np.float32)
    Wq = np.asarray(Wq, dtype=np.float32)
    Wk = np.asarray(Wk, dtype=np.float32)
    Wv = np.asarray(Wv, dtype=np.float32)
    Wk_audio = np.asarray(Wk_audio, dtype=np.float32)
    Wv_audio = np.asarray(Wv_audio, dtype=np.float32)
    Wo = np.asarray(Wo, dtype=np.float32)
    bo = np.asarray(bo, dtype=np.float32)

    nc = get_nc()
    in_maps = _host_prep(hidden_states, encoder_hidden_states, audio_hidden_states,
                         Wq, Wk, Wv, Wk_audio, Wv_audio, Wo)
    res = bass_utils.run_bass_kernel_spmd(nc, in_maps, list(range(N_CORES)))

    out = np.empty((B, S, D), dtype=np.float32)
    core = 0
    for b in range(B):
        for half in range(2):
            out[b, half * SEQ_PER_CORE : (half + 1) * SEQ_PER_CORE] = res.results[core]["out"]
            core += 1
    out += bo[None, None, :]
    return out



# revision 33
# speedup vs baseline: 1.1611x; 1.1611x over previous
"""Trainium2 Bass kernel for AudioAdapterAttnProcessor.

Reference computation (B=4, S=4096, D=1024, H=16, HD=64, C=768,
S_TXT=77, S_AUD=16):
    q = (hidden @ Wq)                                  [B, S, H, HD]
    base  = softmax(q k_t^T / 8) v_t   (text cross-attn, k/v from encoder)
    audio = softmax(q k_a^T / 8) v_a   (audio cross-attn)
    out = concat_heads(base + audio) @ Wo + bo

Sharding: (batch x seq-half) -> 8 cores; each core handles one batch's
2048 queries for all 16 heads.  No collectives: every query row of the
output depends only on its own hidden row (cross-attention to 93 fixed
keys per batch), so the gather is a pure concatenation.

Per-core device program (everything transposed so PE contracts naturally):
    qT   = Wq^T @ hiddenT                (hiddenT fed pre-transposed, bf16)
    s    = qT_pair.T @ kc_blockdiag      natural scores [128q, 186] — ONE
           128-contraction matmul per (head-pair, q-tile); kc is laid out
           block-diagonally on host so both heads' 93-key scores come out
           side by side (guaranteed pair concurrency, no row-tiling bet)
    p    = exp(s/8) (fused scale); denominators via pair-batched DVE
           segmented reduces + reciprocal; normalize via broadcast mult
           into pn [128q, 2h, 4qt, 128k-padded] bf16
    pT   = DMA-transpose(pn)             ONE SBUF->SBUF xbar DMA per pair
           -> expT [128k, 2h, 4qt, 128q]; replaces 8 PE transposes + their
           PSUM->SBUF ACT copy (the xbar maps logical row r to tile r//128,
           partition r%128 with a fixed 128 stride — KPAD must be 128)
    pv   = v_h^T.T @ expT_h              -> attn_outT [64, 512] per head
           (col-tiled pair: out partitions 0-63 / 64-127 run concurrently)
    out  = attn_T.T @ Wo                 natural [128q, 1024] -> DRAM bf16
bo is added on the host during the gather (it is per-output-feature,
which is the free dim of the natural-layout output).

Pipelining: scores of pair g, out-projection of chunk c-1 slot j=g-pv_lag,
q-projection of chunk c+1 slot j, PV of slot j, then the attn_T PSUM->SBUF
copy of slot j-at_lag. The at-copy trails PV so its wait on the DMA-fed
chain cannot back up the ACT FIFO into the exps that recycle the scores
PSUM. ht k-slices of chunk c+2 are spread one per slot to keep the SP
HWDGE queue short; output stores go on the ACT queue. Edge chunks
(first/last) run with pv_lag+lag_edge_extra since they have less PE work
per slot to hide the softmax+DMA chain under.

Designs tried and rejected on hardware: transposed scores with GPSIMD
partition_all_reduce denominators (each reduce costs ~3.6us on real HW
vs 0.8us in the cost model — 64 of them added ~230us/rep), fp8
DoubleRow projections (4-8% rel err vs the 2% tolerance), and moving the
ob/at copies to other engine mixes (obact variant measured +16us).
HW-measured: DMA-transpose design ~90us vs PE-transpose baseline ~105us
(same-session interleaved slope, reps=33).
"""

import sys

sys.path.insert(0, "/opt/trn_rl_repo")

from contextlib import ExitStack

import numpy as np
import ml_dtypes

import concourse.bass as bass
import concourse.mybir as mybir
import concourse.tile as tile
from concourse import bacc
from concourse.masks import make_identity

BF16 = ml_dtypes.bfloat16

B, S, D = 4, 4096, 1024
S_TXT, S_AUD = 77, 16
C = 768
H = 16
HD = 64
NK = S_TXT + S_AUD  # 93 keys after concat
# Keys padded to 128 for the DMA-transpose: the xbar hardware maps logical
# transpose row r to (free-tile r//128, partition r%128) with a FIXED 128
# block stride (measured on HW; CoreSim instead uses the out AP's partition
# size as the stride, so KPAD=96 passes sim but is wrong on silicon).
KPAD = 128
P = 128
SCALE = 1.0 / np.sqrt(HD)  # 0.125

N_CORES = 8
SEQ_PER_CORE = S // 2  # 2048
CHUNK = 512
N_CHUNKS = SEQ_PER_CORE // CHUNK  # 4
KT = D // P  # 8 contraction tiles
QT_PER_CHUNK = CHUNK // P  # 4
NG = H // 2  # 8 head pairs == 8 mt groups == 8 outproj groups


DEFAULT_CFG = dict(hidden=3, qT=2, puT=4, dall=4, pnT=4, expT=5, attn_T=2,
                   out_sb=3, accps=4, sps=2, tps=1, vps=2, pv_lag=3,
                   prio_qT=0, prio_at=0, prio_expT=0, qT_dve=False, at_dve=False,
                   expT_dve=False, ob_dve=True, out_bf16=True, out_dma_act=True,
                   pu_bf16=True, dmat=True, ht_split=True, dmat_act=False,
                   ht_act=False, at_lag=1, ht_spread=True, lag_edge_extra=2,
                   warm_mm=32, bd_scores=True)


INST_TAGS = {}


def _tag(inst, label):
    try:
        INST_TAGS[inst.ins.name] = label
    except Exception:
        pass
    return inst


def build_bass(cfg=None, reps=1):
    """Build the SPMD single-core Bass program (same program on all 8 cores).

    reps > 1 repeats the whole computation back-to-back inside the NEFF
    (same inputs -> same outputs); used only for slope-based timing.
    """
    cfg = {**DEFAULT_CFG, **(cfg or {})}
    INST_TAGS.clear()
    nc = bacc.Bacc("TRN2", target_bir_lowering=False, debug=False, num_devices=N_CORES)

    ht_d = nc.dram_tensor("ht", [P, KT * SEQ_PER_CORE], mybir.dt.bfloat16, kind="ExternalInput")
    wq_d = nc.dram_tensor("wq", [P, KT * D], mybir.dt.bfloat16, kind="ExternalInput")
    wo_d = nc.dram_tensor("wo", [P, KT * D], mybir.dt.bfloat16, kind="ExternalInput")
    kc_w = 2 * NK if (cfg or DEFAULT_CFG).get("bd_scores", DEFAULT_CFG["bd_scores"]) else NK
    kc_d = nc.dram_tensor("kc", [P, NG * kc_w], mybir.dt.bfloat16, kind="ExternalInput")
    v_d = nc.dram_tensor("v", [P, D], mybir.dt.bfloat16, kind="ExternalInput")
    out_dt = mybir.dt.bfloat16 if cfg["out_bf16"] else mybir.dt.float32
    out_d = nc.dram_tensor("out", [SEQ_PER_CORE, D], out_dt, kind="ExternalOutput")

    ht_view = ht_d[:].rearrange("p (k q) -> p k q", k=KT)  # [128, 8, 2048]

    import contextlib

    with tile.TileContext(nc) as tc, ExitStack() as ctx:
        def gate_copy(dst, src, prio, on_dve):
            """PSUM->SBUF copy that gates PE work; optionally boosted/moved."""
            cm = (tc.high_priority(None if prio < 0 else prio)
                  if prio else contextlib.nullcontext())
            with cm:
                if on_dve:
                    nc.vector.tensor_copy(dst, src)
                else:
                    nc.scalar.copy(dst, src)

        wpool = ctx.enter_context(tc.tile_pool(name="weights", bufs=1))
        hpool = ctx.enter_context(tc.tile_pool(name="hidden", bufs=cfg["hidden"]))
        qpool = ctx.enter_context(tc.tile_pool(name="qT", bufs=cfg["qT"]))
        pupool = ctx.enter_context(tc.tile_pool(name="pu", bufs=cfg["puT"]))
        dpool = ctx.enter_context(tc.tile_pool(name="ds", bufs=cfg["dall"]))
        pnpool = ctx.enter_context(tc.tile_pool(name="pn", bufs=cfg["pnT"]))
        epool = ctx.enter_context(tc.tile_pool(name="expT", bufs=cfg["expT"]))
        atpool = ctx.enter_context(tc.tile_pool(name="attn_T", bufs=cfg["attn_T"]))
        opool = ctx.enter_context(tc.tile_pool(name="out_sb", bufs=cfg["out_sb"]))

        accps = ctx.enter_context(tc.tile_pool(name="accps", bufs=cfg["accps"], space="PSUM"))
        sps = ctx.enter_context(tc.tile_pool(name="sps", bufs=cfg["sps"], space="PSUM"))
        tps = (None if cfg["dmat"] else
               ctx.enter_context(tc.tile_pool(name="tps", bufs=cfg["tps"], space="PSUM")))
        vps = ctx.enter_context(tc.tile_pool(name="vps", bufs=cfg["vps"], space="PSUM"))

        if cfg["warm_mm"]:
            # Dummy matmuls with no DMA deps: they run during the initial
            # weight-load wait and flip the PE HAM clock gate to 8/8 before
            # the first real matmul (the busy window needs ~3.4us).
            wsb = wpool.tile([P, 64], mybir.dt.bfloat16)
            nc.vector.memset(wsb[:].rearrange("p a -> p a"), 1.0)
            # full-size tile so the accps pool's per-buffer size is unchanged
            wps = accps.tile([P, CHUNK], mybir.dt.float32, tag="acc")
            for _ in range(cfg["warm_mm"]):
                nc.tensor.matmul(wps[0:64, 0:64], lhsT=wsb[:], rhs=wsb[:],
                                 start=True, stop=True)

        # One-time loads
        wq_t = wpool.tile([P, KT * D], mybir.dt.bfloat16)
        wo_t = wpool.tile([P, KT * D], mybir.dt.bfloat16)
        kc_t = wpool.tile([P, NG * kc_w], mybir.dt.bfloat16)
        v_t = wpool.tile([P, D], mybir.dt.bfloat16)
        ident = None if cfg["dmat"] else wpool.tile([P, P], mybir.dt.bfloat16)
        # DMA issue order matters: the HWDGE queue drains in order, so put
        # everything the first q-projection needs ahead of the 2MB wo load.
        # Per-k-tile splits let matmuls start on sub-tile deps.
        ht_t0 = hpool.tile([P, KT, CHUNK], mybir.dt.bfloat16, tag="ht")
        for kt in range(KT):
            nc.sync.dma_start(wq_t[:, kt * D : (kt + 1) * D], wq_d[:, kt * D : (kt + 1) * D])
            nc.sync.dma_start(ht_t0[:, kt, :], ht_view[:, kt, 0:CHUNK])
        nc.sync.dma_start(kc_t[:], kc_d[:])
        nc.sync.dma_start(v_t[:], v_d[:])
        for kt in range(KT):
            nc.sync.dma_start(wo_t[:, kt * D : (kt + 1) * D], wo_d[:, kt * D : (kt + 1) * D])
        if ident is not None:
            make_identity(nc, ident[:])

        def emit_ht(pos_chunk, tile_=None):
            ht_t = tile_ or hpool.tile([P, KT, CHUNK], mybir.dt.bfloat16, tag="ht")
            if tile_ is None:
                hq = nc.scalar if cfg["ht_act"] else nc.sync
                if cfg["ht_split"]:
                    # per-kt slices bound the HWDGE queue delay for the
                    # probs DMA-transposes that share the queue
                    for kt in range(KT):
                        hq.dma_start(
                            ht_t[:, kt, :],
                            ht_view[:, kt, pos_chunk * CHUNK : (pos_chunk + 1) * CHUNK],
                        )
                else:
                    hq.dma_start(
                        ht_t[:], ht_view[:, :, pos_chunk * CHUNK : (pos_chunk + 1) * CHUNK]
                    )
            return ht_t

        def emit_qproj_mt(ht_t, qT_t, mt):
            qp = accps.tile([P, CHUNK], mybir.dt.float32, tag="acc")
            for kt in range(KT):
                _tag(nc.tensor.matmul(
                    qp[:],
                    lhsT=wq_t[:, kt * D + mt * P : kt * D + (mt + 1) * P],
                    rhs=ht_t[:, kt, :],
                    start=(kt == 0),
                    stop=(kt == KT - 1),
                ), f"qproj")
            gate_copy(qT_t[:, mt, :], qp[:], cfg["prio_qT"], cfg["qT_dve"])

        def emit_scores_pair(qT_t, g):
            """Natural-layout scores + softmax for both heads of pair g.

            Per pair: 8 PE matmuls (ap=93), 2 ACT exps (one per head, into a
            shared pair tile), then pair-batched DVE: 2 segmented reduces,
            1 reciprocal, 2 normalizes.  Returns the bf16 probs tile
            pn [P, 2, QT, NK] (natural layout, q on partitions).
            """
            pu_dt = mybir.dt.bfloat16 if cfg["pu_bf16"] else mybir.dt.float32
            pu = pupool.tile([P, 2, QT_PER_CHUNK, NK], pu_dt)
            if cfg["bd_scores"]:
                # kc holds the pair's keys block-diagonally: rows 0-63 are
                # head 2g features over key-cols 0-92, rows 64-127 are head
                # 2g+1 over cols 93-185, so ONE 128-contraction matmul per
                # q-tile yields both heads' scores side by side.
                for half in range(2):
                    sp = sps.tile([P, 2, 2 * NK], mybir.dt.float32)
                    for q2 in range(2):
                        qt = 2 * half + q2
                        _tag(nc.tensor.matmul(
                            sp[:, q2, :],
                            lhsT=qT_t[:, g, qt * P : (qt + 1) * P],
                            rhs=kc_t[:, g * 2 * NK : (g + 1) * 2 * NK],
                            start=True,
                            stop=True,
                        ), "scores")
                    nc.scalar.activation(
                        pu[:, :, 2 * half : 2 * half + 2, :]
                        .rearrange("p h q k -> p q h k"),
                        sp[:].rearrange("p q (h k) -> p q h k", h=2),
                        mybir.ActivationFunctionType.Exp,
                        scale=float(SCALE),
                    )
            else:
                for hh in range(2):
                    off = hh * HD
                    sp = sps.tile([P, QT_PER_CHUNK * NK], mybir.dt.float32)
                    for qt in range(QT_PER_CHUNK):
                        _tag(nc.tensor.matmul(
                            sp[:, qt * NK : (qt + 1) * NK],
                            lhsT=qT_t[off : off + HD, g, qt * P : (qt + 1) * P],
                            rhs=kc_t[off : off + HD, g * NK : (g + 1) * NK],
                            start=True,
                            stop=True,
                        ), "scores")
                    nc.scalar.activation(
                        pu[:, hh, :, :].rearrange("p q k -> p (q k)"),
                        sp[:],
                        mybir.ActivationFunctionType.Exp,
                        scale=float(SCALE),
                    )
            # 16 denominators per pair: 2 heads x 4 q-tiles x (text, audio)
            ds = dpool.tile([P, 2, QT_PER_CHUNK, 2], mybir.dt.float32)
            nc.vector.reduce_sum(
                ds[:, :, :, 0], pu[:, :, :, 0:S_TXT], axis=mybir.AxisListType.X
            )
            nc.vector.reduce_sum(
                ds[:, :, :, 1], pu[:, :, :, S_TXT:NK], axis=mybir.AxisListType.X
            )
            nc.vector.reciprocal(
                ds[:].rearrange("p a b c -> p (a b c)"),
                ds[:].rearrange("p a b c -> p (a b c)"),
            )
            kw = KPAD if cfg["dmat"] else NK
            pn = pnpool.tile([P, 2, QT_PER_CHUNK, kw], mybir.dt.bfloat16)
            if cfg["dmat"]:
                # pad columns must hold finite bytes for the DMA-transpose
                nc.gpsimd.memset(pn[:, :, :, NK:KPAD], 0.0)
            nc.vector.tensor_tensor(
                pn[:, :, :, 0:S_TXT],
                pu[:, :, :, 0:S_TXT],
                ds[:, :, :, 0, None].to_broadcast([P, 2, QT_PER_CHUNK, S_TXT]),
                mybir.AluOpType.mult,
            )
            nc.vector.tensor_tensor(
                pn[:, :, :, S_TXT:NK],
                pu[:, :, :, S_TXT:NK],
                ds[:, :, :, 1, None].to_broadcast([P, 2, QT_PER_CHUNK, S_AUD]),
                mybir.AluOpType.mult,
            )
            if not cfg["dmat"]:
                return pn
            # One SBUF->SBUF DMA-transpose turns the whole pair's probs
            # [128q, (2h, 4qt, 96k)] into expT [96k, (2h, 4qt), 128q] off the
            # PE/ACT critical engines (xbar: 8x6 16x128-tiles, ~0.7us).
            expT = epool.tile([KPAD, 2, QT_PER_CHUNK, P], mybir.dt.bfloat16)
            dq = nc.scalar if cfg["dmat_act"] else nc.sync
            _tag(dq.dma_start_transpose(
                expT[:], pn[:].rearrange("p a b c -> p (a b c)")
            ), "dmat")
            return expT

        def emit_pv_pair(g, probs, at_t):
            """PV matmuls for the pair into its attn_T rows. `probs` is the
            DMA-transposed expT tile (dmat) or the natural-layout pn tile
            (legacy PE-transpose path)."""
            if cfg["dmat"]:
                expT = probs
            else:
                tpp = tps.tile([NK, 2, CHUNK], mybir.dt.bfloat16)
                for hh in range(2):
                    for qt in range(QT_PER_CHUNK):
                        nc.tensor.transpose(
                            tpp[:, hh, qt * P : (qt + 1) * P],
                            probs[:, hh, qt, :],
                            ident[:],
                        )
                expT = epool.tile([NK, 2, CHUNK], mybir.dt.bfloat16)
                gate_copy(expT[:], tpp[:], cfg["prio_expT"], cfg["expT_dve"])
            pvp = vps.tile([P, CHUNK], mybir.dt.float32)
            for hh in range(2):
                h = 2 * g + hh
                off = hh * HD
                _tag(nc.tensor.matmul(
                    pvp[off : off + HD, :],
                    lhsT=v_t[0:NK, h * HD : (h + 1) * HD],
                    rhs=(expT[0:NK, hh, :, :] if cfg["dmat"] else expT[0:NK, hh, :]),
                    start=True,
                    stop=True,
                ), "pv")
            return pvp

        def emit_at_copy(g, pvp, at_t):
            # attn_T tile g holds heads 2g (rows 0-63) and 2g+1 (64-127).
            # Deferred a slot behind the PV matmuls (at_lag) so this copy's
            # wait on the DMA-fed PV chain cannot back up its engine's FIFO
            # into the exp instructions that recycle the scores PSUM.
            gate_copy(at_t[:, g, :], pvp[:], cfg["prio_at"], cfg["at_dve"])

        def emit_outproj_group(c, at_t, j):
            qt, nb = j // 2, j % 2
            op = accps.tile([P, CHUNK], mybir.dt.float32, tag="acc")
            for kt in range(KT):
                _tag(nc.tensor.matmul(
                    op[:],
                    lhsT=at_t[:, kt, qt * P : (qt + 1) * P],
                    rhs=wo_t[:, kt * D + nb * CHUNK : kt * D + (nb + 1) * CHUNK],
                    start=(kt == 0),
                    stop=(kt == KT - 1),
                ), "outproj")
            ob = opool.tile([P, CHUNK], out_dt)
            gate_copy(ob[:], op[:], 0, cfg["ob_dve"])
            out_q = nc.scalar if cfg["out_dma_act"] else nc.sync
            out_q.dma_start(
                out_d[
                    c * CHUNK + qt * P : c * CHUNK + (qt + 1) * P,
                    nb * CHUNK : (nb + 1) * CHUNK,
                ],
                ob[:],
            )

        chunks = [c for _ in range(reps) for c in range(N_CHUNKS)]
        n = len(chunks)
        ht_tiles = {0: ht_t0}
        if n > 1:
            ht_tiles[1] = emit_ht(chunks[1])

        # Prologue: q-projection of the first chunk.
        qT_t = qpool.tile([P, KT, CHUNK], mybir.dt.bfloat16)
        for mt in range(KT):
            emit_qproj_mt(ht_tiles[0], qT_t, mt)

        at_prev = None
        c_prev = None
        at_lag = cfg["at_lag"]  # extra slots the PSUM->SBUF attn copy trails PV
        for i, c in enumerate(chunks):
            # fill/tail chunks interleave less PE work per slot, so the
            # softmax+DMA chain needs more slots of lag to stay hidden
            lag = cfg["pv_lag"]
            if at_prev is None or i + 1 >= n:
                lag = min(NG - 1, lag + cfg["lag_edge_extra"])
            ht_spread = cfg["ht_spread"] and i + 2 < n
            ht_nxt2 = None
            if i + 2 < n:
                if ht_spread:
                    ht_nxt2 = hpool.tile([P, KT, CHUNK], mybir.dt.bfloat16, tag="ht")
                    ht_tiles[i + 2] = ht_nxt2
                else:
                    ht_tiles[i + 2] = emit_ht(chunks[i + 2])
            ht_nxt = ht_tiles.get(i + 1)
            ht_tiles.pop(i, None)
            qT_next = (qpool.tile([P, KT, CHUNK], mybir.dt.bfloat16, name="qT_next")
                       if ht_nxt is not None else None)
            at_t = atpool.tile([P, KT, CHUNK], mybir.dt.bfloat16, tag="at")
            # PV trails scores by PV_LAG pair-slots so the softmax+DMA chain
            # latency hides under interleaved outproj/qproj work.
            probs_q = {}
            pend_at = {}

            def run_slot(j, at_t=at_t, c_prev=c_prev, at_prev=at_prev,
                         qT_next=qT_next, ht_nxt=ht_nxt, probs_q=probs_q,
                         pend_at=pend_at):
                if 0 <= j < NG:
                    if at_prev is not None:
                        emit_outproj_group(c_prev, at_prev, j)
                    if qT_next is not None:
                        emit_qproj_mt(ht_nxt, qT_next, j)
                    pend_at[j] = emit_pv_pair(j, probs_q.pop(j), at_t)
                jj = j - at_lag
                if jj in pend_at:
                    emit_at_copy(jj, pend_at.pop(jj), at_t)

            for g in range(NG + lag + at_lag):
                if g < NG:
                    probs_q[g] = emit_scores_pair(qT_t, g)
                    if ht_spread:
                        # one ht k-slice of chunk i+2 per slot keeps the SP
                        # HWDGE queue short for the probs DMA-transposes
                        nc.sync.dma_start(
                            ht_nxt2[:, g, :],
                            ht_view[:, g, chunks[i + 2] * CHUNK
                                    : (chunks[i + 2] + 1) * CHUNK],
                        )
                run_slot(g - lag)
            at_prev, c_prev = at_t, c
            if qT_next is not None:
                qT_t = qT_next
        # Epilogue: outproj of the final chunk.
        for j in range(2 * QT_PER_CHUNK):
            emit_outproj_group(c_prev, at_prev, j)

    nc.compile()
    nc._audio_cfg = cfg
    return nc


def _host_prep(hidden_states, encoder_hidden_states, audio_hidden_states,
               Wq, Wk, Wv, Wk_audio, Wv_audio, Wo, bd_scores=None):
    """Build the per-core input maps (all layouts pre-arranged on host)."""
    if bd_scores is None:
        bd_scores = DEFAULT_CFG["bd_scores"]
    wq_sb = np.ascontiguousarray(
        Wq.reshape(KT, P, D).transpose(1, 0, 2).reshape(P, KT * D)
    ).astype(BF16)
    wo_sb = np.ascontiguousarray(
        Wo.reshape(KT, P, D).transpose(1, 0, 2).reshape(P, KT * D)
    ).astype(BF16)

    in_maps = []
    for b in range(B):
        # kv projections for this batch: tiny, done on host
        k_full = np.concatenate(
            [encoder_hidden_states[b] @ Wk, audio_hidden_states[b] @ Wk_audio], axis=0
        )  # [93, 1024]
        v_full = np.concatenate(
            [encoder_hidden_states[b] @ Wv, audio_hidden_states[b] @ Wv_audio], axis=0
        )  # [93, 1024]
        kc_pair = k_full.T.reshape(NG, P, NK).transpose(1, 0, 2)  # [P, pair, key]
        if bd_scores:
            # block-diagonal pair layout: head 2g keys in cols 0:93 live on
            # feature rows 0:64, head 2g+1 keys in cols 93:186 on rows 64:128
            kc_bd = np.zeros((P, NG, 2 * NK), np.float32)
            kc_bd[0:HD, :, 0:NK] = kc_pair[0:HD]
            kc_bd[HD:P, :, NK : 2 * NK] = kc_pair[HD:P]
            kc_sb = np.ascontiguousarray(kc_bd.reshape(P, NG * 2 * NK)).astype(BF16)
        else:
            kc_sb = np.ascontiguousarray(kc_pair.reshape(P, NG * NK)).astype(BF16)
        v_sb = np.zeros((P, D), dtype=BF16)
        v_sb[:NK] = v_full.astype(BF16)

        for half in range(2):
            rows = hidden_states[b, half * SEQ_PER_CORE : (half + 1) * SEQ_PER_CORE]
            ht_sb = np.ascontiguousarray(
                rows.T.reshape(KT, P, SEQ_PER_CORE)
                .transpose(1, 0, 2)
                .reshape(P, KT * SEQ_PER_CORE)
            ).astype(BF16)
            in_maps.append(
                {"ht": ht_sb, "wq": wq_sb, "wo": wo_sb, "kc": kc_sb, "v": v_sb}
            )
    return in_maps


_NC_CACHE = {}


def get_nc():
    if "nc" not in _NC_CACHE:
        _NC_CACHE["nc"] = build_bass()
    return _NC_CACHE["nc"]


def kernel(hidden_states, encoder_hidden_states, audio_hidden_states,
           Wq, Wk, Wv, Wk_audio, Wv_audio, Wo, bo):
    from concourse import bass_utils

    hidden_states = np.asarray(hidden_states, dtype=np.float32)
    encoder_hidden_states = np.asarray(encoder_hidden_states, dtype=np.float32)
    audio_hidden_states = np.asarray(audio_hidden_states, dtype=np.float32)
    Wq = np.asarray(Wq, dtype=np.float32)
    Wk = np.asarray(Wk, dtype=np.float32)
    Wv = np.asarray(Wv, dtype=np.float32)
    Wk_audio = np.asarray(Wk_audio, dtype=np.float32)
    Wv_audio = np.asarray(Wv_audio, dtype=np.float32)
    Wo = np.asarray(Wo, dtype=np.float32)
    bo = np.asarray(bo, dtype=np.float32)

    nc = get_nc()
    in_maps = _host_prep(hidden_states, encoder_hidden_states, audio_hidden_states,
                         Wq, Wk, Wv, Wk_audio, Wv_audio, Wo)
    res = bass_utils.run_bass_kernel_spmd(nc, in_maps, list(range(N_CORES)))

    out = np.empty((B, S, D), dtype=np.float32)
    core = 0
    for b in range(B):
        for half in range(2):
            out[b, half * SEQ_PER_CORE : (half + 1) * SEQ_PER_CORE] = res.results[core]["out"]
            core += 1
    out += bo[None, None, :]
    return out

